# revision 15
# baseline (speedup 1.0000x reference)
"""Trainium2 Bass kernel for nn_BasisAffinityGAT (B=8, N=512, D=R=128, K=8).

Math (matches reference.py):
    fused = concat(desc, nve) @ W_fuse + b_fuse                 [B,N,D]
    q = fused @ W_q[k];  kk = fused @ W_k[k]                    per basis
    e_q[b,k,n] = lrelu(q).a_q[k];  e_k likewise
    logits = e_q[:,:,:,None] + e_k[:,:,None,:], symmetrized
    alpha  = softmax(logits, -1); ema update; bias_log = log(clip(ema'))

Exact algebra used:
  * sym-logits[i,j] = 0.5*(s_i + s_j) with s = e_q + e_k, so the row
    softmax collapses: alpha[b,k,i,j] = softmax_j(0.5*s[b,k,:])[j],
    independent of i.
  * 0.5*s[b,k,n] = (0.5 a_q[k]) . lrelu(q) + (0.5 a_k[k]) . lrelu(kk)
    with the fusion layer folded on host:
    q = x @ (W_fuse @ W_q[k]) + (b_fuse @ W_q[k]), x = concat(desc,nve).
  * bias_log content is batch-independent ([K,N,N] broadcast over B);
    with alpha_ema == 0 (what setup_inputs produces) every bias row is
    identical, so ONE [128,N] tile feeds the whole 8 MiB bias output
    via two broadcast DMAs. Nonzero alpha_ema falls back to exact
    host-side bias computation from the device alpha.
  * exp runs UNSHIFTED in fp16: the host verifies 0.5*s is inside a
    safe window (exact, cheap) and falls back to a max-shifted build
    (reduce_max negate=True into the Exp bias) otherwise.

Sharding: core m owns basis k=m for all batches (no collectives).

Performance structure (output-bandwidth-bound: 16 MiB of writes/core;
one hw queue sustains ~283 GB/s but two together reach ~390 GB/s):
  * alpha writes alternate between the sync and Act hw queues; the
    bias tail is split across both.
  * all matmuls fp16 (1 PE pass); x staged fp16; all small weights in
    ONE packed fp16 tensor (separate gpsimd SWDGE loads cost ~5us).
  * engine balance per batch (~2.3us cadence):
    PE 7 matmuls | Act: Prelu(q), Exp+accum | DVE: lrelu(kk) via
    (0.2x max x), PSUM->SBUF copy, EMA accumulate, reciprocal |
    gpsimd: 1/sum row replicate. (gpsimd cannot touch PSUM.)
  * batch b's softmax/broadcast tail is emitted inside batch b+1's
    matmul stream (1-deep software pipeline).
"""

import sys

import numpy as np

if "/opt/trn_rl_repo" not in sys.path:
    sys.path.insert(0, "/opt/trn_rl_repo")

from contextlib import ExitStack

import concourse.bass as bass
import concourse.tile as tile
from concourse import bacc, mybir
from concourse.bass_utils import run_bass_kernel_spmd

B, N, D, K = 8, 512, 128, 8
R = D
MOM = 0.99
EPS = 1e-6
N_CORES = 8
F32 = mybir.dt.float32
F16 = mybir.dt.float16
AF = mybir.ActivationFunctionType
AX = mybir.AxisListType
ALU = mybir.AluOpType

# weight pack layout (columns of wcat [D, 4R+2]):
#   [0:2R) Wq' = W_fuse@W_q[m] | [2R:4R) Wk' | [4R] 0.5 a_q | [4R+1] 0.5 a_k
WCOLS = 4 * R + 2

# |0.5*s| beyond this switches to the max-shifted build (fp16 exp safety;
# also keeps exp underflow irrelevant vs the 1e-6 EMA clip)
SAFE_HALF_S = 9.0


def build(zero_bias=True, shift=False):
    """SPMD per-core Bass program (identical on all 8 cores); ema==0 path."""
    nc = bacc.Bacc("TRN2", target_bir_lowering=False, debug=False,
                   num_devices=N_CORES)

    xT = nc.dram_tensor("xT", [B, D, 2 * N], F16, kind="ExternalInput")
    wcat = nc.dram_tensor("wcat", [D, WCOLS], F16, kind="ExternalInput")
    if not zero_bias:
        bqk = nc.dram_tensor("bqk", [D, 2], F32, kind="ExternalInput")
    alpha = nc.dram_tensor("alpha", [B, N, N], F32, kind="ExternalOutput")
    biaso = nc.dram_tensor("bias", [B, N, N], F32, kind="ExternalOutput")

    with ExitStack() as ctx:
        tc = ctx.enter_context(tile.TileContext(nc))
        const = ctx.enter_context(tc.tile_pool(name="const", bufs=1))
        xpool = ctx.enter_context(tc.tile_pool(name="xpool", bufs=1))
        work = ctx.enter_context(tc.tile_pool(name="work", bufs=2))
        psum = ctx.enter_context(tc.tile_pool(name="psum", bufs=1, space="PSUM"))

        wcat_sb = const.tile([D, WCOLS], F16)
        ones_sb = const.tile([1, D], F16)
        pbs_acc = const.tile([128, N], F32)
        nc.vector.memset(ones_sb[:], 1.0)
        if not zero_bias:
            bqk_sb = const.tile([D, 2], F32)

        # all reads up front on the sync hw queue (weights first — they
        # gate everything); writes will alternate sync/Act queues.
        nc.sync.dma_start(wcat_sb[:], wcat[:])
        if not zero_bias:
            nc.sync.dma_start(bqk_sb[:], bqk[:])
        xbs = []
        for b in range(B):
            xb = xpool.tile([D, 2 * N], F16, tag="xb", bufs=B)
            if b == 0:
                # split: batch 0's first matmul only needs the low half
                nc.sync.dma_start(xb[:, 0:N], xT[0][:, 0:N])
                nc.sync.dma_start(xb[:, N:2 * N], xT[0][:, N:2 * N])
            else:
                nc.sync.dma_start(xb[:], xT[b])
            xbs.append(xb)

        wq0, wq1 = wcat_sb[:, 0:R], wcat_sb[:, R:2 * R]
        wk0, wk1 = wcat_sb[:, 2 * R:3 * R], wcat_sb[:, 3 * R:4 * R]
        aqh = wcat_sb[:, 4 * R:4 * R + 1]
        akh = wcat_sb[:, 4 * R + 1:4 * R + 2]

        def lrelu(dst, src, which):  # Act engine (only PE/Act/DVE see PSUM,
            # and DVE ops can read PSUM at most once -> no 1-op lrelu there)
            if zero_bias:
                nc.scalar.activation(dst, src, AF.Prelu, alpha=0.2)
            else:
                nc.scalar.activation(dst, src, AF.Prelu, alpha=0.2,
                                     bias=bqk_sb[:, which:which + 1])

        prev = None  # 1-deep pipeline state of batch b-1

        def emit_bcast(st):
            prep = psum.tile([128, N], F32, tag="rep", bufs=2)
            nc.tensor.matmul(prep[:], st["rrep"][:], st["expv"][:],
                             start=True, stop=True)
            st["prep"] = prep

        def emit_flush(st):
            b, prep = st["b"], st["prep"]
            # stage the row TWICE so the alpha DMA uses 4 KB descriptors
            # (2 KB descriptors cap a queue well below the DMA bus rate)
            rep_t = work.tile([128, 2 * N], F32, tag="rept", bufs=4)
            nc.vector.tensor_copy(
                rep_t[:].rearrange("p (c n) -> p c n", c=2),
                prep[:].rearrange("p (o n) -> p o n",
                                  o=1).broadcast_to([128, 2, N]))
            if b == 0:
                nc.vector.tensor_scalar_mul(pbs_acc[:], prep[:],
                                            (1.0 - MOM) / B / MOM)
            else:
                nc.vector.scalar_tensor_tensor(
                    pbs_acc[:], prep[:], (1.0 - MOM) / B / MOM, pbs_acc[:],
                    op0=ALU.mult, op1=ALU.add)
            st["rep_t"] = rep_t

        def emit_alpha_dma(st):
            b, rep_t = st["b"], st["rep_t"]
            src = rep_t[:].rearrange(
                "p (o m) -> p o m", o=1).broadcast_to([128, 2, 2 * N])
            dst = alpha[b].rearrange("(p c w) j -> p c (w j)", p=128, c=2)
            eng = nc.sync if b % 2 == 0 else nc.scalar
            eng.dma_start(dst, src)

        for b in range(B):
            xb = xbs[b]
            pq = psum.tile([D, N], F32, tag="mm", bufs=4)
            nc.tensor.matmul(pq[:], wq0, xb[:, 0:N], start=True, stop=False)
            nc.tensor.matmul(pq[:], wq1, xb[:, N:2 * N],
                             start=False, stop=True)
            pk = psum.tile([D, N], F32, tag="mm", bufs=4)
            nc.tensor.matmul(pk[:], wk0, xb[:, 0:N], start=True, stop=False)
            nc.tensor.matmul(pk[:], wk1, xb[:, N:2 * N],
                             start=False, stop=True)
            if prev is not None:
                emit_bcast(prev)
            lq = work.tile([D, N], F16, tag="lq", bufs=4)
            lrelu(lq[:], pq[:], 0)
            lk = work.tile([D, N], F16, tag="lk", bufs=4)
            lrelu(lk[:], pk[:], 1)
            if prev is not None:
                emit_flush(prev)
                if prev["b"] % 2 == 0:
                    emit_alpha_dma(prev)
            ps = psum.tile([1, N], F32, tag="ps", bufs=2)
            nc.tensor.matmul(ps[:], aqh, lq[:], start=True, stop=False)
            nc.tensor.matmul(ps[:], akh, lk[:], start=False, stop=True)

            expv = work.tile([1, N], F16, tag="ex", bufs=4)
            sume = work.tile([1, 1], F32, tag="se", bufs=8)
            if shift:
                negm = work.tile([1, 1], F32, tag="negm", bufs=8)
                nc.vector.reduce_max(negm[:], ps[:], axis=AX.X, negate=True)
                nc.scalar.activation(expv[:], ps[:], AF.Exp, bias=negm[:],
                                     accum_out=sume[:])
            else:
                nc.scalar.activation(expv[:], ps[:], AF.Exp,
                                     accum_out=sume[:])
            if prev is not None and prev["b"] % 2 == 1:
                emit_alpha_dma(prev)  # on Act queue, after exp(b)
            rsum = work.tile([1, 1], F32, tag="rs", bufs=8)
            nc.vector.reciprocal(rsum[:], sume[:])
            rrep = work.tile([1, D], F16, tag="rr", bufs=4)
            nc.vector.tensor_scalar_mul(rrep[:], ones_sb[:], rsum[:])
            prev = dict(expv=expv, rrep=rrep, b=b)
            if b == 0:
                # flush batch 0 eagerly at max priority: its alpha opens
                # the write stream, so its latency is the kernel's head.
                with tc.high_priority():
                    emit_bcast(prev)
                    emit_flush(prev)
                    emit_alpha_dma(prev)
                prev = None

        emit_bcast(prev)
        emit_flush(prev)
        emit_alpha_dma(prev)

        # ---- bias (ema == 0): one tile, every output row identical -----
        v = work.tile([128, N], F32, tag="v", bufs=1)
        nc.vector.tensor_scalar_max(v[:], pbs_acc[:], EPS / MOM)
        bias_t = work.tile([128, N], F32, tag="biassb", bufs=1)
        nc.scalar.activation(bias_t[:], v[:], AF.Ln, scale=MOM)
        src = bias_t[:].rearrange(
            "p (o n) -> p o n", o=1).broadcast_to([128, 4 * B // 2, N])
        dst = biaso.ap().rearrange("b (x p) j -> p (b x) j", p=128)
        nc.scalar.dma_start(dst[:, 0:16, :], src)
        nc.sync.dma_start(dst[:, 16:32, :], src)

    nc.compile()
    return nc


_NC_CACHE = {}


def _get_nc(zero_bias=True, shift=False):
    key = (zero_bias, shift)
    if key not in _NC_CACHE:
        _NC_CACHE[key] = build(zero_bias, shift)
    return _NC_CACHE[key]


def _needs_shift(X, W_fuse, b_fuse, W_q, W_k, a_q, a_k):
    """Exact host check: is 0.5*s inside the fp16-safe exp window?"""
    hi = 0.0
    for m in range(K):
        q = X @ (W_fuse @ W_q[m]) + b_fuse @ W_q[m]
        kk = X @ (W_fuse @ W_k[m]) + b_fuse @ W_k[m]
        s2 = 0.5 * (np.where(q > 0, q, 0.2 * q) @ a_q[m]
                    + np.where(kk > 0, kk, 0.2 * kk) @ a_k[m])
        hi = max(hi, float(np.abs(s2).max()))
    return hi > SAFE_HALF_S


def make_in_maps(desc_embeddings, name_value_embeddings, W_fuse, b_fuse,
                 W_q, W_k, a, alpha_ema, zero_bias=True):
    """Host-side sharding / weight prep -> per-core input dicts."""
    desc = np.asarray(desc_embeddings, np.float32)
    nve = np.asarray(name_value_embeddings, np.float32)
    W_fuse = np.asarray(W_fuse, np.float32)
    b_fuse = np.asarray(b_fuse, np.float32)
    W_q = np.asarray(W_q, np.float32)
    W_k = np.asarray(W_k, np.float32)
    a = np.asarray(a, np.float32)

    a_q = a[:, :R, 0]                      # [K,R]
    a_k = a[:, R:, 0]                      # [K,R]
    wq_f = np.einsum("cd,kdr->kcr", W_fuse, W_q)         # [K,2D,R]
    wk_f = np.einsum("cd,kdr->kcr", W_fuse, W_k)
    bq = np.einsum("d,kdr->kr", b_fuse, W_q)             # [K,R]
    bk = np.einsum("d,kdr->kr", b_fuse, W_k)

    # xT[b] fp16 [D, 2N]: partition d = [desc[b].T[d,:], nve[b].T[d,:]]
    xT = np.ascontiguousarray(
        np.stack([np.concatenate([desc[b].T, nve[b].T], axis=1)
                  for b in range(B)], axis=0).astype(np.float16))

    def lhsT(w):  # [2D, M] -> [D, 2M]: [:, h*M:(h+1)*M] = w[h*D:(h+1)*D]
        M = w.shape[1]
        return w.reshape(2, D, M).transpose(1, 0, 2).reshape(D, 2 * M)

    in_maps = []
    for m in range(N_CORES):
        wc = np.concatenate(
            [lhsT(wq_f[m]), lhsT(wk_f[m]),
             (0.5 * a_q[m]).reshape(D, 1), (0.5 * a_k[m]).reshape(D, 1)],
            axis=1).astype(np.float16)
        im = dict(xT=xT, wcat=np.ascontiguousarray(wc))
        if not zero_bias:
            im["bqk"] = np.ascontiguousarray(
                np.stack([bq[m], bk[m]], axis=1).astype(np.float32))
        in_maps.append(im)
    return in_maps


def gather(results, alpha_ema=None):
    alpha_full = np.stack([r["alpha"] for r in results], axis=1)
    ema = None if alpha_ema is None else np.asarray(alpha_ema, np.float32)
    if ema is not None and np.any(ema):
        # general-EMA fallback: exact host-side bias from device alpha
        new_ema = MOM * ema + (1.0 - MOM) * alpha_full.mean(axis=0)
        bias1 = np.log(np.maximum(new_ema, EPS))
        bias_full = np.ascontiguousarray(
            np.broadcast_to(bias1[None], (B, K, N, N)).astype(np.float32))
    else:
        bias_full = np.stack([r["bias"] for r in results], axis=1)
    return bias_full, alpha_full


def kernel(**inputs):
    desc = np.asarray(inputs["desc_embeddings"], np.float32)
    nve = np.asarray(inputs["name_value_embeddings"], np.float32)
    X = np.concatenate([desc, nve], axis=-1).reshape(B * N, 2 * D)
    a = np.asarray(inputs["a"], np.float32)
    zero_bias = not np.any(np.asarray(inputs["b_fuse"]))
    shift = _needs_shift(
        X, np.asarray(inputs["W_fuse"], np.float32),
        np.asarray(inputs["b_fuse"], np.float32),
        np.asarray(inputs["W_q"], np.float32),
        np.asarray(inputs["W_k"], np.float32),
        a[:, :R, 0], a[:, R:, 0])
    nc = _get_nc(zero_bias, shift)
    in_maps = make_in_maps(**inputs, zero_bias=zero_bias)
    res = run_bass_kernel_spmd(nc, in_maps, list(range(N_CORES)))
    return gather(res.results, inputs.get("alpha_ema"))


# revision 17
# speedup vs baseline: 1.0258x; 1.0258x over previous
"""Trainium2 Bass kernel for nn_BasisAffinityGAT (B=8, N=512, D=R=128, K=8).

Math (matches reference.py):
    fused = concat(desc, nve) @ W_fuse + b_fuse                 [B,N,D]
    q = fused @ W_q[k];  kk = fused @ W_k[k]                    per basis
    e_q[b,k,n] = lrelu(q).a_q[k];  e_k likewise
    logits = e_q[:,:,:,None] + e_k[:,:,None,:], symmetrized
    alpha  = softmax(logits, -1); ema update; bias_log = log(clip(ema'))

Exact algebra used:
  * sym-logits[i,j] = 0.5*(s_i + s_j) with s = e_q + e_k, so the row
    softmax collapses: alpha[b,k,i,j] = softmax_j(0.5*s[b,k,:])[j],
    independent of i.
  * 0.5*s[b,k,n] = (0.5 a_q[k]) . lrelu(q) + (0.5 a_k[k]) . lrelu(kk)
    with the fusion layer folded on host:
    q = x @ (W_fuse @ W_q[k]) + (b_fuse @ W_q[k]), x = concat(desc,nve).
  * bias_log content is batch-independent ([K,N,N] broadcast over B);
    with alpha_ema == 0 (what setup_inputs produces) every bias row is
    identical, so ONE [128,N] tile feeds the whole 8 MiB bias output
    via two broadcast DMAs. Nonzero alpha_ema falls back to exact
    host-side bias computation from the device alpha.
  * exp runs UNSHIFTED in fp16: the host verifies 0.5*s is inside a
    safe window (exact, cheap) and falls back to a max-shifted build
    (reduce_max negate=True into the Exp bias) otherwise.

Sharding: core m owns basis k=m for all batches (no collectives).

Performance structure (output-bandwidth-bound: 16 MiB of writes/core;
one hw queue sustains ~283 GB/s but two together reach ~390 GB/s):
  * alpha writes alternate between the sync and Act hw queues; the
    bias tail is split across both.
  * all matmuls fp16 (1 PE pass); x staged fp16; all small weights in
    ONE packed fp16 tensor (separate gpsimd SWDGE loads cost ~5us).
  * engine balance per batch (~2.3us cadence):
    PE 7 matmuls | Act: Prelu(q), Exp+accum | DVE: lrelu(kk) via
    (0.2x max x), PSUM->SBUF copy, EMA accumulate, reciprocal |
    gpsimd: 1/sum row replicate. (gpsimd cannot touch PSUM.)
  * batch b's softmax/broadcast tail is emitted inside batch b+1's
    matmul stream (1-deep software pipeline).
"""

import sys

import numpy as np

if "/opt/trn_rl_repo" not in sys.path:
    sys.path.insert(0, "/opt/trn_rl_repo")

from contextlib import ExitStack

import concourse.bass as bass
import concourse.tile as tile
from concourse import bacc, mybir
from concourse.bass_utils import run_bass_kernel_spmd

B, N, D, K = 8, 512, 128, 8
R = D
MOM = 0.99
EPS = 1e-6
N_CORES = 8
F32 = mybir.dt.float32
F16 = mybir.dt.float16
AF = mybir.ActivationFunctionType
AX = mybir.AxisListType
ALU = mybir.AluOpType

# weight pack layout (columns of wcat [D, 4R+2]):
#   [0:2R) Wq' = W_fuse@W_q[m] | [2R:4R) Wk' | [4R] 0.5 a_q | [4R+1] 0.5 a_k
WCOLS = 4 * R + 2

# |0.5*s| beyond this switches to the max-shifted build (fp16 exp safety;
# also keeps exp underflow irrelevant vs the 1e-6 EMA clip)
SAFE_HALF_S = 9.0


def build(zero_bias=True, shift=False):
    """SPMD per-core Bass program (identical on all 8 cores); ema==0 path."""
    nc = bacc.Bacc("TRN2", target_bir_lowering=False, debug=False,
                   num_devices=N_CORES)

    xT = nc.dram_tensor("xT", [B, D, 2 * N], F16, kind="ExternalInput")
    wcat = nc.dram_tensor("wcat", [D, WCOLS], F16, kind="ExternalInput")
    if not zero_bias:
        bqk = nc.dram_tensor("bqk", [D, 2], F32, kind="ExternalInput")
    alpha = nc.dram_tensor("alpha", [B, N, N], F32, kind="ExternalOutput")
    biaso = nc.dram_tensor("bias", [B, N, N], F32, kind="ExternalOutput")

    with ExitStack() as ctx:
        tc = ctx.enter_context(tile.TileContext(nc))
        const = ctx.enter_context(tc.tile_pool(name="const", bufs=1))
        xpool = ctx.enter_context(tc.tile_pool(name="xpool", bufs=1))
        work = ctx.enter_context(tc.tile_pool(name="work", bufs=2))
        psum = ctx.enter_context(tc.tile_pool(name="psum", bufs=1, space="PSUM"))

        wcat_sb = const.tile([D, WCOLS], F16)
        ones_sb = const.tile([1, D], F16)
        pbs_acc = const.tile([128, N], F32)
        nc.vector.memset(ones_sb[:], 1.0)
        if not zero_bias:
            bqk_sb = const.tile([D, 2], F32)

        # all reads up front on the sync hw queue (weights first — they
        # gate everything); writes will alternate sync/Act queues.
        nc.sync.dma_start(wcat_sb[:], wcat[:])
        if not zero_bias:
            nc.sync.dma_start(bqk_sb[:], bqk[:])
        xbs = []
        for b in range(B):
            xb = xpool.tile([D, 2 * N], F16, tag="xb", bufs=B)
            if b == 0:
                # split: batch 0's first matmul only needs the low half
                nc.sync.dma_start(xb[:, 0:N], xT[0][:, 0:N])
                nc.sync.dma_start(xb[:, N:2 * N], xT[0][:, N:2 * N])
            else:
                nc.sync.dma_start(xb[:], xT[b])
            xbs.append(xb)

        wq0, wq1 = wcat_sb[:, 0:R], wcat_sb[:, R:2 * R]
        wk0, wk1 = wcat_sb[:, 2 * R:3 * R], wcat_sb[:, 3 * R:4 * R]
        aqh = wcat_sb[:, 4 * R:4 * R + 1]
        akh = wcat_sb[:, 4 * R + 1:4 * R + 2]

        def lrelu(dst, src, which):  # Act engine (only PE/Act/DVE see PSUM,
            # and DVE ops can read PSUM at most once -> no 1-op lrelu there)
            if zero_bias:
                nc.scalar.activation(dst, src, AF.Prelu, alpha=0.2)
            else:
                nc.scalar.activation(dst, src, AF.Prelu, alpha=0.2,
                                     bias=bqk_sb[:, which:which + 1])

        prev = None  # 1-deep pipeline state of batch b-1

        def emit_bcast(st):
            prep = psum.tile([128, N], F32, tag="rep", bufs=2)
            nc.tensor.matmul(prep[:], st["rrep"][:], st["expv"][:],
                             start=True, stop=True)
            st["prep"] = prep

        def emit_flush(st):
            b, prep = st["b"], st["prep"]
            rep_t = work.tile([128, N], F32, tag="rept", bufs=4)
            nc.vector.tensor_copy(rep_t[:], prep[:])
            if b == 0:
                nc.vector.tensor_scalar_mul(pbs_acc[:], prep[:],
                                            (1.0 - MOM) / B / MOM)
            else:
                nc.vector.scalar_tensor_tensor(
                    pbs_acc[:], prep[:], (1.0 - MOM) / B / MOM, pbs_acc[:],
                    op0=ALU.mult, op1=ALU.add)
            st["rep_t"] = rep_t

        def emit_alpha_dma(st):
            b, rep_t = st["b"], st["rep_t"]
            src = rep_t[:].rearrange(
                "p (o n) -> p o n", o=1).broadcast_to([128, 4, N])
            dst = alpha[b].rearrange("(p i) j -> p i j", p=128)
            eng = nc.sync if b % 2 == 0 else nc.scalar
            eng.dma_start(dst, src)

        for b in range(B):
            xb = xbs[b]
            pq = psum.tile([D, N], F32, tag="mm", bufs=4)
            nc.tensor.matmul(pq[:], wq0, xb[:, 0:N], start=True, stop=False)
            nc.tensor.matmul(pq[:], wq1, xb[:, N:2 * N],
                             start=False, stop=True)
            pk = psum.tile([D, N], F32, tag="mm", bufs=4)
            nc.tensor.matmul(pk[:], wk0, xb[:, 0:N], start=True, stop=False)
            nc.tensor.matmul(pk[:], wk1, xb[:, N:2 * N],
                             start=False, stop=True)
            if prev is not None:
                emit_bcast(prev)
            lq = work.tile([D, N], F16, tag="lq", bufs=4)
            lrelu(lq[:], pq[:], 0)
            lk = work.tile([D, N], F16, tag="lk", bufs=4)
            lrelu(lk[:], pk[:], 1)
            if prev is not None:
                emit_flush(prev)
                if prev["b"] % 2 == 0:
                    emit_alpha_dma(prev)
            ps = psum.tile([1, N], F32, tag="ps", bufs=2)
            nc.tensor.matmul(ps[:], aqh, lq[:], start=True, stop=False)
            nc.tensor.matmul(ps[:], akh, lk[:], start=False, stop=True)

            expv = work.tile([1, N], F16, tag="ex", bufs=4)
            sume = work.tile([1, 1], F32, tag="se", bufs=8)
            if shift:
                negm = work.tile([1, 1], F32, tag="negm", bufs=8)
                nc.vector.reduce_max(negm[:], ps[:], axis=AX.X, negate=True)
                nc.scalar.activation(expv[:], ps[:], AF.Exp, bias=negm[:],
                                     accum_out=sume[:])
            else:
                nc.scalar.activation(expv[:], ps[:], AF.Exp,
                                     accum_out=sume[:])
            if prev is not None and prev["b"] % 2 == 1:
                emit_alpha_dma(prev)  # on Act queue, after exp(b)
            rsum = work.tile([1, 1], F32, tag="rs", bufs=8)
            nc.vector.reciprocal(rsum[:], sume[:])
            rrep = work.tile([1, D], F16, tag="rr", bufs=4)
            nc.vector.tensor_scalar_mul(rrep[:], ones_sb[:], rsum[:])
            prev = dict(expv=expv, rrep=rrep, b=b)
            if b == 0:
                # flush batch 0 eagerly at max priority: its alpha opens
                # the write stream, so its latency is the kernel's head.
                with tc.high_priority():
                    emit_bcast(prev)
                    emit_flush(prev)
                    emit_alpha_dma(prev)
                prev = None

        emit_bcast(prev)
        emit_flush(prev)
        emit_alpha_dma(prev)

        # ---- bias (ema == 0): one tile, every output row identical -----
        v = work.tile([128, N], F32, tag="v", bufs=1)
        nc.vector.tensor_scalar_max(v[:], pbs_acc[:], EPS / MOM)
        bias_t = work.tile([128, N], F32, tag="biassb", bufs=1)
        nc.scalar.activation(bias_t[:], v[:], AF.Ln, scale=MOM)
        src = bias_t[:].rearrange(
            "p (o n) -> p o n", o=1).broadcast_to([128, 4 * B // 2, N])
        dst = biaso.ap().rearrange("b (x p) j -> p (b x) j", p=128)
        nc.scalar.dma_start(dst[:, 0:16, :], src)
        nc.sync.dma_start(dst[:, 16:32, :], src)

    nc.compile()
    return nc


_NC_CACHE = {}


def _get_nc(zero_bias=True, shift=False):
    key = (zero_bias, shift)
    if key not in _NC_CACHE:
        _NC_CACHE[key] = build(zero_bias, shift)
    return _NC_CACHE[key]


def _needs_shift(X, W_fuse, b_fuse, W_q, W_k, a_q, a_k):
    """Exact host check: is 0.5*s inside the fp16-safe exp window?"""
    hi = 0.0
    for m in range(K):
        q = X @ (W_fuse @ W_q[m]) + b_fuse @ W_q[m]
        kk = X @ (W_fuse @ W_k[m]) + b_fuse @ W_k[m]
        s2 = 0.5 * (np.where(q > 0, q, 0.2 * q) @ a_q[m]
                    + np.where(kk > 0, kk, 0.2 * kk) @ a_k[m])
        hi = max(hi, float(np.abs(s2).max()))
    return hi > SAFE_HALF_S


def make_in_maps(desc_embeddings, name_value_embeddings, W_fuse, b_fuse,
                 W_q, W_k, a, alpha_ema, zero_bias=True):
    """Host-side sharding / weight prep -> per-core input dicts."""
    desc = np.asarray(desc_embeddings, np.float32)
    nve = np.asarray(name_value_embeddings, np.float32)
    W_fuse = np.asarray(W_fuse, np.float32)
    b_fuse = np.asarray(b_fuse, np.float32)
    W_q = np.asarray(W_q, np.float32)
    W_k = np.asarray(W_k, np.float32)
    a = np.asarray(a, np.float32)

    a_q = a[:, :R, 0]                      # [K,R]
    a_k = a[:, R:, 0]                      # [K,R]
    wq_f = np.einsum("cd,kdr->kcr", W_fuse, W_q)         # [K,2D,R]
    wk_f = np.einsum("cd,kdr->kcr", W_fuse, W_k)
    bq = np.einsum("d,kdr->kr", b_fuse, W_q)             # [K,R]
    bk = np.einsum("d,kdr->kr", b_fuse, W_k)

    # xT[b] fp16 [D, 2N]: partition d = [desc[b].T[d,:], nve[b].T[d,:]]
    xT = np.ascontiguousarray(
        np.stack([np.concatenate([desc[b].T, nve[b].T], axis=1)
                  for b in range(B)], axis=0).astype(np.float16))

    def lhsT(w):  # [2D, M] -> [D, 2M]: [:, h*M:(h+1)*M] = w[h*D:(h+1)*D]
        M = w.shape[1]
        return w.reshape(2, D, M).transpose(1, 0, 2).reshape(D, 2 * M)

    in_maps = []
    for m in range(N_CORES):
        wc = np.concatenate(
            [lhsT(wq_f[m]), lhsT(wk_f[m]),
             (0.5 * a_q[m]).reshape(D, 1), (0.5 * a_k[m]).reshape(D, 1)],
            axis=1).astype(np.float16)
        im = dict(xT=xT, wcat=np.ascontiguousarray(wc))
        if not zero_bias:
            im["bqk"] = np.ascontiguousarray(
                np.stack([bq[m], bk[m]], axis=1).astype(np.float32))
        in_maps.append(im)
    return in_maps


def gather(results, alpha_ema=None):
    alpha_full = np.stack([r["alpha"] for r in results], axis=1)
    ema = None if alpha_ema is None else np.asarray(alpha_ema, np.float32)
    if ema is not None and np.any(ema):
        # general-EMA fallback: exact host-side bias from device alpha
        new_ema = MOM * ema + (1.0 - MOM) * alpha_full.mean(axis=0)
        bias1 = np.log(np.maximum(new_ema, EPS))
        bias_full = np.ascontiguousarray(
            np.broadcast_to(bias1[None], (B, K, N, N)).astype(np.float32))
    else:
        bias_full = np.stack([r["bias"] for r in results], axis=1)
    return bias_full, alpha_full


def kernel(**inputs):
    desc = np.asarray(inputs["desc_embeddings"], np.float32)
    nve = np.asarray(inputs["name_value_embeddings"], np.float32)
    X = np.concatenate([desc, nve], axis=-1).reshape(B * N, 2 * D)
    a = np.asarray(inputs["a"], np.float32)
    zero_bias = not np.any(np.asarray(inputs["b_fuse"]))
    shift = _needs_shift(
        X, np.asarray(inputs["W_fuse"], np.float32),
        np.asarray(inputs["b_fuse"], np.float32),
        np.asarray(inputs["W_q"], np.float32),
        np.asarray(inputs["W_k"], np.float32),
        a[:, :R, 0], a[:, R:, 0])
    nc = _get_nc(zero_bias, shift)
    in_maps = make_in_maps(**inputs, zero_bias=zero_bias)
    res = run_bass_kernel_spmd(nc, in_maps, list(range(N_CORES)))
    return gather(res.results, inputs.get("alpha_ema"))


# revision 18
# speedup vs baseline: 1.0322x; 1.0062x over previous
"""Trainium2 Bass kernel for nn_BasisAffinityGAT (B=8, N=512, D=R=128, K=8).

Math (matches reference.py):
    fused = concat(desc, nve) @ W_fuse + b_fuse                 [B,N,D]
    q = fused @ W_q[k];  kk = fused @ W_k[k]                    per basis
    e_q[b,k,n] = lrelu(q).a_q[k];  e_k likewise
    logits = e_q[:,:,:,None] + e_k[:,:,None,:], symmetrized
    alpha  = softmax(logits, -1); ema update; bias_log = log(clip(ema'))

Exact algebra used:
  * sym-logits[i,j] = 0.5*(s_i + s_j) with s = e_q + e_k, so the row
    softmax collapses: alpha[b,k,i,j] = softmax_j(0.5*s[b,k,:])[j],
    independent of i.
  * 0.5*s[b,k,n] = (0.5 a_q[k]) . lrelu(q) + (0.5 a_k[k]) . lrelu(kk)
    with the fusion layer folded on host:
    q = x @ (W_fuse @ W_q[k]) + (b_fuse @ W_q[k]), x = concat(desc,nve).
  * bias_log content is batch-independent ([K,N,N] broadcast over B);
    with alpha_ema == 0 (what setup_inputs produces) every bias row is
    identical, so ONE [128,N] tile feeds the whole 8 MiB bias output
    via two broadcast DMAs. Nonzero alpha_ema falls back to exact
    host-side bias computation from the device alpha.
  * exp runs UNSHIFTED in fp16: the host verifies 0.5*s is inside a
    safe window (exact, cheap) and falls back to a max-shifted build
    (reduce_max negate=True into the Exp bias) otherwise.

Sharding: core m owns basis k=m for all batches (no collectives).

Performance structure (output-bandwidth-bound: 16 MiB of writes/core;
one hw queue sustains ~283 GB/s but two together reach ~390 GB/s):
  * alpha writes alternate between the sync and Act hw queues; the
    bias tail is split across both.
  * all matmuls fp16 (1 PE pass); x staged fp16; all small weights in
    ONE packed fp16 tensor (separate gpsimd SWDGE loads cost ~5us).
  * engine balance per batch (~2.3us cadence):
    PE 7 matmuls | Act: Prelu(q), Exp+accum | DVE: lrelu(kk) via
    (0.2x max x), PSUM->SBUF copy, EMA accumulate, reciprocal |
    gpsimd: 1/sum row replicate. (gpsimd cannot touch PSUM.)
  * batch b's softmax/broadcast tail is emitted inside batch b+1's
    matmul stream (1-deep software pipeline).
"""

import sys

import numpy as np

if "/opt/trn_rl_repo" not in sys.path:
    sys.path.insert(0, "/opt/trn_rl_repo")

from contextlib import ExitStack

import concourse.bass as bass
import concourse.tile as tile
from concourse import bacc, mybir
from concourse.bass_utils import run_bass_kernel_spmd

B, N, D, K = 8, 512, 128, 8
R = D
MOM = 0.99
EPS = 1e-6
N_CORES = 8
F32 = mybir.dt.float32
F16 = mybir.dt.float16
AF = mybir.ActivationFunctionType
AX = mybir.AxisListType
ALU = mybir.AluOpType

# weight pack layout (columns of wcat [D, 4R+2]):
#   [0:2R) Wq' = W_fuse@W_q[m] | [2R:4R) Wk' | [4R] 0.5 a_q | [4R+1] 0.5 a_k
WCOLS = 4 * R + 2

# |0.5*s| beyond this switches to the max-shifted build (fp16 exp safety;
# also keeps exp underflow irrelevant vs the 1e-6 EMA clip)
SAFE_HALF_S = 9.0


def build(zero_bias=True, shift=False):
    """SPMD per-core Bass program (identical on all 8 cores); ema==0 path."""
    nc = bacc.Bacc("TRN2", target_bir_lowering=False, debug=False,
                   num_devices=N_CORES)

    xT = nc.dram_tensor("xT", [B, D, 2 * N], F16, kind="ExternalInput")
    wcat = nc.dram_tensor("wcat", [D, WCOLS], F16, kind="ExternalInput")
    if not zero_bias:
        bqk = nc.dram_tensor("bqk", [D, 2], F32, kind="ExternalInput")
    alpha = nc.dram_tensor("alpha", [B, N, N], F32, kind="ExternalOutput")
    biaso = nc.dram_tensor("bias", [B, N, N], F32, kind="ExternalOutput")

    with ExitStack() as ctx:
        tc = ctx.enter_context(tile.TileContext(nc))
        const = ctx.enter_context(tc.tile_pool(name="const", bufs=1))
        xpool = ctx.enter_context(tc.tile_pool(name="xpool", bufs=1))
        work = ctx.enter_context(tc.tile_pool(name="work", bufs=2))
        psum = ctx.enter_context(tc.tile_pool(name="psum", bufs=1, space="PSUM"))

        wcat_sb = const.tile([D, WCOLS], F16)
        ones_sb = const.tile([1, D], F16)
        pbs_acc = const.tile([128, N], F32)
        nc.vector.memset(ones_sb[:], 1.0)
        if not zero_bias:
            bqk_sb = const.tile([D, 2], F32)

        # all reads up front on the sync hw queue (weights first — they
        # gate everything); writes will alternate sync/Act queues.
        nc.sync.dma_start(wcat_sb[:], wcat[:])
        if not zero_bias:
            nc.sync.dma_start(bqk_sb[:], bqk[:])
        xbs = []
        for b in range(B):
            xb = xpool.tile([D, 2 * N], F16, tag="xb", bufs=B)
            if b == 0:
                # split batch 0 across BOTH queues: the scalar queue's
                # first doorbell beats the sync queue (which carries the
                # weights first), so xb0 lands ~1us earlier.
                nc.sync.dma_start(xb[:, 0:N], xT[0][:, 0:N])
                nc.scalar.dma_start(xb[:, N:2 * N], xT[0][:, N:2 * N])
            else:
                nc.sync.dma_start(xb[:], xT[b])
            xbs.append(xb)

        wq0, wq1 = wcat_sb[:, 0:R], wcat_sb[:, R:2 * R]
        wk0, wk1 = wcat_sb[:, 2 * R:3 * R], wcat_sb[:, 3 * R:4 * R]
        aqh = wcat_sb[:, 4 * R:4 * R + 1]
        akh = wcat_sb[:, 4 * R + 1:4 * R + 2]

        def lrelu(dst, src, which):  # Act engine (only PE/Act/DVE see PSUM,
            # and DVE ops can read PSUM at most once -> no 1-op lrelu there)
            if zero_bias:
                nc.scalar.activation(dst, src, AF.Prelu, alpha=0.2)
            else:
                nc.scalar.activation(dst, src, AF.Prelu, alpha=0.2,
                                     bias=bqk_sb[:, which:which + 1])

        prev = None  # 1-deep pipeline state of batch b-1

        def emit_bcast(st):
            prep = psum.tile([128, N], F32, tag="rep", bufs=2)
            nc.tensor.matmul(prep[:], st["rrep"][:], st["expv"][:],
                             start=True, stop=True)
            st["prep"] = prep

        def emit_flush(st):
            b, prep = st["b"], st["prep"]
            rep_t = work.tile([128, N], F32, tag="rept", bufs=4)
            nc.vector.tensor_copy(rep_t[:], prep[:])
            if b == 0:
                nc.vector.tensor_scalar_mul(pbs_acc[:], prep[:],
                                            (1.0 - MOM) / B / MOM)
            else:
                nc.vector.scalar_tensor_tensor(
                    pbs_acc[:], prep[:], (1.0 - MOM) / B / MOM, pbs_acc[:],
                    op0=ALU.mult, op1=ALU.add)
            st["rep_t"] = rep_t

        def emit_alpha_dma(st):
            b, rep_t = st["b"], st["rep_t"]
            src = rep_t[:].rearrange(
                "p (o n) -> p o n", o=1).broadcast_to([128, 4, N])
            dst = alpha[b].rearrange("(p i) j -> p i j", p=128)
            eng = nc.sync if b % 2 == 0 else nc.scalar
            eng.dma_start(dst, src)

        for b in range(B):
            xb = xbs[b]
            pq = psum.tile([D, N], F32, tag="mm", bufs=4)
            nc.tensor.matmul(pq[:], wq0, xb[:, 0:N], start=True, stop=False)
            nc.tensor.matmul(pq[:], wq1, xb[:, N:2 * N],
                             start=False, stop=True)
            pk = psum.tile([D, N], F32, tag="mm", bufs=4)
            nc.tensor.matmul(pk[:], wk0, xb[:, 0:N], start=True, stop=False)
            nc.tensor.matmul(pk[:], wk1, xb[:, N:2 * N],
                             start=False, stop=True)
            if prev is not None:
                emit_bcast(prev)
            lq = work.tile([D, N], F16, tag="lq", bufs=4)
            lrelu(lq[:], pq[:], 0)
            lk = work.tile([D, N], F16, tag="lk", bufs=4)
            lrelu(lk[:], pk[:], 1)
            if prev is not None:
                emit_flush(prev)
                if prev["b"] % 2 == 0:
                    emit_alpha_dma(prev)
            ps = psum.tile([1, N], F32, tag="ps", bufs=2)
            nc.tensor.matmul(ps[:], aqh, lq[:], start=True, stop=False)
            nc.tensor.matmul(ps[:], akh, lk[:], start=False, stop=True)

            expv = work.tile([1, N], F16, tag="ex", bufs=4)
            sume = work.tile([1, 1], F32, tag="se", bufs=8)
            if shift:
                negm = work.tile([1, 1], F32, tag="negm", bufs=8)
                nc.vector.reduce_max(negm[:], ps[:], axis=AX.X, negate=True)
                nc.scalar.activation(expv[:], ps[:], AF.Exp, bias=negm[:],
                                     accum_out=sume[:])
            else:
                nc.scalar.activation(expv[:], ps[:], AF.Exp,
                                     accum_out=sume[:])
            if prev is not None and prev["b"] % 2 == 1:
                emit_alpha_dma(prev)  # on Act queue, after exp(b)
            rsum = work.tile([1, 1], F32, tag="rs", bufs=8)
            nc.vector.reciprocal(rsum[:], sume[:])
            rrep = work.tile([1, D], F16, tag="rr", bufs=4)
            nc.vector.tensor_scalar_mul(rrep[:], ones_sb[:], rsum[:])
            prev = dict(expv=expv, rrep=rrep, b=b)
            if b == 0:
                # flush batch 0 eagerly at max priority: its alpha opens
                # the write stream, so its latency is the kernel's head.
                with tc.high_priority():
                    emit_bcast(prev)
                    emit_flush(prev)
                    emit_alpha_dma(prev)
                prev = None

        emit_bcast(prev)
        emit_flush(prev)
        emit_alpha_dma(prev)

        # ---- bias (ema == 0): one tile, every output row identical -----
        v = work.tile([128, N], F32, tag="v", bufs=1)
        nc.vector.tensor_scalar_max(v[:], pbs_acc[:], EPS / MOM)
        bias_t = work.tile([128, N], F32, tag="biassb", bufs=1)
        nc.scalar.activation(bias_t[:], v[:], AF.Ln, scale=MOM)
        src = bias_t[:].rearrange(
            "p (o n) -> p o n", o=1).broadcast_to([128, 4 * B // 2, N])
        dst = biaso.ap().rearrange("b (x p) j -> p (b x) j", p=128)
        nc.scalar.dma_start(dst[:, 0:16, :], src)
        nc.sync.dma_start(dst[:, 16:32, :], src)

    nc.compile()
    return nc


_NC_CACHE = {}


def _get_nc(zero_bias=True, shift=False):
    key = (zero_bias, shift)
    if key not in _NC_CACHE:
        _NC_CACHE[key] = build(zero_bias, shift)
    return _NC_CACHE[key]


def _needs_shift(X, W_fuse, b_fuse, W_q, W_k, a_q, a_k):
    """Exact host check: is 0.5*s inside the fp16-safe exp window?"""
    hi = 0.0
    for m in range(K):
        q = X @ (W_fuse @ W_q[m]) + b_fuse @ W_q[m]
        kk = X @ (W_fuse @ W_k[m]) + b_fuse @ W_k[m]
        s2 = 0.5 * (np.where(q > 0, q, 0.2 * q) @ a_q[m]
                    + np.where(kk > 0, kk, 0.2 * kk) @ a_k[m])
        hi = max(hi, float(np.abs(s2).max()))
    return hi > SAFE_HALF_S


def make_in_maps(desc_embeddings, name_value_embeddings, W_fuse, b_fuse,
                 W_q, W_k, a, alpha_ema, zero_bias=True):
    """Host-side sharding / weight prep -> per-core input dicts."""
    desc = np.asarray(desc_embeddings, np.float32)
    nve = np.asarray(name_value_embeddings, np.float32)
    W_fuse = np.asarray(W_fuse, np.float32)
    b_fuse = np.asarray(b_fuse, np.float32)
    W_q = np.asarray(W_q, np.float32)
    W_k = np.asarray(W_k, np.float32)
    a = np.asarray(a, np.float32)

    a_q = a[:, :R, 0]                      # [K,R]
    a_k = a[:, R:, 0]                      # [K,R]
    wq_f = np.einsum("cd,kdr->kcr", W_fuse, W_q)         # [K,2D,R]
    wk_f = np.einsum("cd,kdr->kcr", W_fuse, W_k)
    bq = np.einsum("d,kdr->kr", b_fuse, W_q)             # [K,R]
    bk = np.einsum("d,kdr->kr", b_fuse, W_k)

    # xT[b] fp16 [D, 2N]: partition d = [desc[b].T[d,:], nve[b].T[d,:]]
    xT = np.ascontiguousarray(
        np.stack([np.concatenate([desc[b].T, nve[b].T], axis=1)
                  for b in range(B)], axis=0).astype(np.float16))

    def lhsT(w):  # [2D, M] -> [D, 2M]: [:, h*M:(h+1)*M] = w[h*D:(h+1)*D]
        M = w.shape[1]
        return w.reshape(2, D, M).transpose(1, 0, 2).reshape(D, 2 * M)

    in_maps = []
    for m in range(N_CORES):
        wc = np.concatenate(
            [lhsT(wq_f[m]), lhsT(wk_f[m]),
             (0.5 * a_q[m]).reshape(D, 1), (0.5 * a_k[m]).reshape(D, 1)],
            axis=1).astype(np.float16)
        im = dict(xT=xT, wcat=np.ascontiguousarray(wc))
        if not zero_bias:
            im["bqk"] = np.ascontiguousarray(
                np.stack([bq[m], bk[m]], axis=1).astype(np.float32))
        in_maps.append(im)
    return in_maps


def gather(results, alpha_ema=None):
    alpha_full = np.stack([r["alpha"] for r in results], axis=1)
    ema = None if alpha_ema is None else np.asarray(alpha_ema, np.float32)
    if ema is not None and np.any(ema):
        # general-EMA fallback: exact host-side bias from device alpha
        new_ema = MOM * ema + (1.0 - MOM) * alpha_full.mean(axis=0)
        bias1 = np.log(np.maximum(new_ema, EPS))
        bias_full = np.ascontiguousarray(
            np.broadcast_to(bias1[None], (B, K, N, N)).astype(np.float32))
    else:
        bias_full = np.stack([r["bias"] for r in results], axis=1)
    return bias_full, alpha_full


def kernel(**inputs):
    desc = np.asarray(inputs["desc_embeddings"], np.float32)
    nve = np.asarray(inputs["name_value_embeddings"], np.float32)
    X = np.concatenate([desc, nve], axis=-1).reshape(B * N, 2 * D)
    a = np.asarray(inputs["a"], np.float32)
    zero_bias = not np.any(np.asarray(inputs["b_fuse"]))
    shift = _needs_shift(
        X, np.asarray(inputs["W_fuse"], np.float32),
        np.asarray(inputs["b_fuse"], np.float32),
        np.asarray(inputs["W_q"], np.float32),
        np.asarray(inputs["W_k"], np.float32),
        a[:, :R, 0], a[:, R:, 0])
    nc = _get_nc(zero_bias, shift)
    in_maps = make_in_maps(**inputs, zero_bias=zero_bias)
    res = run_bass_kernel_spmd(nc, in_maps, list(range(N_CORES)))
    return gather(res.results, inputs.get("alpha_ema"))


# revision 20
# speedup vs baseline: 1.0345x; 1.0023x over previous
"""Trainium2 Bass kernel for nn_BasisAffinityGAT (B=8, N=512, D=R=128, K=8).

Math (matches reference.py):
    fused = concat(desc, nve) @ W_fuse + b_fuse                 [B,N,D]
    q = fused @ W_q[k];  kk = fused @ W_k[k]                    per basis
    e_q[b,k,n] = lrelu(q).a_q[k];  e_k likewise
    logits = e_q[:,:,:,None] + e_k[:,:,None,:], symmetrized
    alpha  = softmax(logits, -1); ema update; bias_log = log(clip(ema'))

Exact algebra used:
  * sym-logits[i,j] = 0.5*(s_i + s_j) with s = e_q + e_k, so the row
    softmax collapses: alpha[b,k,i,j] = softmax_j(0.5*s[b,k,:])[j],
    independent of i.
  * 0.5*s[b,k,n] = (0.5 a_q[k]) . lrelu(q) + (0.5 a_k[k]) . lrelu(kk)
    with the fusion layer folded on host:
    q = x @ (W_fuse @ W_q[k]) + (b_fuse @ W_q[k]), x = concat(desc,nve).
  * bias_log content is batch-independent ([K,N,N] broadcast over B);
    with alpha_ema == 0 (what setup_inputs produces) every bias row is
    identical, so ONE [128,N] tile feeds the whole 8 MiB bias output
    via two broadcast DMAs. Nonzero alpha_ema falls back to exact
    host-side bias computation from the device alpha.
  * exp runs UNSHIFTED in fp16: the host verifies 0.5*s is inside a
    safe window (exact, cheap) and falls back to a max-shifted build
    (reduce_max negate=True into the Exp bias) otherwise.

Sharding: core m owns basis k=m for all batches (no collectives).

Performance structure (output-bandwidth-bound: 16 MiB of writes/core;
one hw queue sustains ~283 GB/s but two together reach ~390 GB/s):
  * alpha writes alternate between the sync and Act hw queues; the
    bias tail is split across both.
  * all matmuls fp16 (1 PE pass); x staged fp16; all small weights in
    ONE packed fp16 tensor (separate gpsimd SWDGE loads cost ~5us).
  * engine balance per batch (~2.3us cadence):
    PE 7 matmuls | Act: Prelu(q), Exp+accum | DVE: lrelu(kk) via
    (0.2x max x), PSUM->SBUF copy, EMA accumulate, reciprocal |
    gpsimd: 1/sum row replicate. (gpsimd cannot touch PSUM.)
  * batch b's softmax/broadcast tail is emitted inside batch b+1's
    matmul stream (1-deep software pipeline).
"""

import sys

import numpy as np

if "/opt/trn_rl_repo" not in sys.path:
    sys.path.insert(0, "/opt/trn_rl_repo")

from contextlib import ExitStack

import concourse.bass as bass
import concourse.tile as tile
from concourse import bacc, mybir
from concourse.bass_utils import run_bass_kernel_spmd

B, N, D, K = 8, 512, 128, 8
R = D
MOM = 0.99
EPS = 1e-6
N_CORES = 8
F32 = mybir.dt.float32
F16 = mybir.dt.float16
AF = mybir.ActivationFunctionType
AX = mybir.AxisListType
ALU = mybir.AluOpType

# weight pack layout (columns of wcat [D, 4R+2]):
#   [0:2R) Wq' = W_fuse@W_q[m] | [2R:4R) Wk' | [4R] 0.5 a_q | [4R+1] 0.5 a_k
WCOLS = 4 * R + 2

# |0.5*s| beyond this switches to the max-shifted build (fp16 exp safety;
# also keeps exp underflow irrelevant vs the 1e-6 EMA clip)
SAFE_HALF_S = 9.0


def build(zero_bias=True, shift=False):
    """SPMD per-core Bass program (identical on all 8 cores); ema==0 path."""
    nc = bacc.Bacc("TRN2", target_bir_lowering=False, debug=False,
                   num_devices=N_CORES)

    xT = nc.dram_tensor("xT", [B, D, 2 * N], F16, kind="ExternalInput")
    wcat = nc.dram_tensor("wcat", [D, WCOLS], F16, kind="ExternalInput")
    if not zero_bias:
        bqk = nc.dram_tensor("bqk", [D, 2], F32, kind="ExternalInput")
    alpha = nc.dram_tensor("alpha", [B, N, N], F32, kind="ExternalOutput")
    biaso = nc.dram_tensor("bias", [B, N, N], F32, kind="ExternalOutput")

    with ExitStack() as ctx:
        tc = ctx.enter_context(tile.TileContext(nc))
        const = ctx.enter_context(tc.tile_pool(name="const", bufs=1))
        xpool = ctx.enter_context(tc.tile_pool(name="xpool", bufs=1))
        work = ctx.enter_context(tc.tile_pool(name="work", bufs=2))
        psum = ctx.enter_context(tc.tile_pool(name="psum", bufs=1, space="PSUM"))

        wcat_sb = const.tile([D, WCOLS], F16)
        ones_sb = const.tile([1, D], F16)
        pbs_acc = const.tile([128, N], F32)
        nc.vector.memset(ones_sb[:], 1.0)
        if not zero_bias:
            bqk_sb = const.tile([D, 2], F32)

        # all reads up front on the sync hw queue (weights first — they
        # gate everything); writes will alternate sync/Act queues.
        nc.sync.dma_start(wcat_sb[:], wcat[:])
        if not zero_bias:
            nc.sync.dma_start(bqk_sb[:], bqk[:])
        xbs = []
        for b in range(B):
            xb = xpool.tile([D, 2 * N], F16, tag="xb", bufs=B)
            if b == 0:
                # split batch 0 across BOTH queues: the scalar queue's
                # first doorbell beats the sync queue (which carries the
                # weights first), so xb0 lands ~1us earlier.
                nc.sync.dma_start(xb[:, 0:N], xT[0][:, 0:N])
                nc.scalar.dma_start(xb[:, N:2 * N], xT[0][:, N:2 * N])
            else:
                nc.sync.dma_start(xb[:], xT[b])
            xbs.append(xb)

        wq0, wq1 = wcat_sb[:, 0:R], wcat_sb[:, R:2 * R]
        wk0, wk1 = wcat_sb[:, 2 * R:3 * R], wcat_sb[:, 3 * R:4 * R]
        aqh = wcat_sb[:, 4 * R:4 * R + 1]
        akh = wcat_sb[:, 4 * R + 1:4 * R + 2]

        def lrelu(dst, src, which):  # Act engine (only PE/Act/DVE see PSUM,
            # and DVE ops can read PSUM at most once -> no 1-op lrelu there)
            if zero_bias:
                nc.scalar.activation(dst, src, AF.Prelu, alpha=0.2)
            else:
                nc.scalar.activation(dst, src, AF.Prelu, alpha=0.2,
                                     bias=bqk_sb[:, which:which + 1])

        prev = None  # 1-deep pipeline state of batch b-1

        def emit_bcast(st):
            prep = psum.tile([128, N], F32, tag="rep", bufs=2)
            nc.tensor.matmul(prep[:], st["rrep"][:], st["expv"][:],
                             start=True, stop=True)
            st["prep"] = prep

        def emit_flush(st):
            b, prep = st["b"], st["prep"]
            rep_t = work.tile([128, N], F32, tag="rept", bufs=4)
            nc.vector.tensor_copy(rep_t[:], prep[:])
            if b == 0:
                nc.vector.tensor_scalar_mul(pbs_acc[:], prep[:],
                                            (1.0 - MOM) / B / MOM)
            else:
                nc.vector.scalar_tensor_tensor(
                    pbs_acc[:], prep[:], (1.0 - MOM) / B / MOM, pbs_acc[:],
                    op0=ALU.mult, op1=ALU.add)
            st["rep_t"] = rep_t

        def emit_alpha_dma(st):
            b, rep_t = st["b"], st["rep_t"]
            src = rep_t[:].rearrange(
                "p (o n) -> p o n", o=1).broadcast_to([128, 4, N])
            dst = alpha[b].rearrange("(p i) j -> p i j", p=128)
            # alpha7 rides sync so the tail drains evenly (see bias split)
            eng = nc.sync if (b % 2 == 0 or b == 7) else nc.scalar
            eng.dma_start(dst, src)

        for b in range(B):
            xb = xbs[b]
            pq = psum.tile([D, N], F32, tag="mm", bufs=4)
            nc.tensor.matmul(pq[:], wq0, xb[:, 0:N], start=True, stop=False)
            nc.tensor.matmul(pq[:], wq1, xb[:, N:2 * N],
                             start=False, stop=True)
            pk = psum.tile([D, N], F32, tag="mm", bufs=4)
            nc.tensor.matmul(pk[:], wk0, xb[:, 0:N], start=True, stop=False)
            nc.tensor.matmul(pk[:], wk1, xb[:, N:2 * N],
                             start=False, stop=True)
            if prev is not None:
                emit_bcast(prev)
            lq = work.tile([D, N], F16, tag="lq", bufs=4)
            lrelu(lq[:], pq[:], 0)
            lk = work.tile([D, N], F16, tag="lk", bufs=4)
            lrelu(lk[:], pk[:], 1)
            if prev is not None:
                emit_flush(prev)
                if prev["b"] % 2 == 0:
                    emit_alpha_dma(prev)
            ps = psum.tile([1, N], F32, tag="ps", bufs=2)
            nc.tensor.matmul(ps[:], aqh, lq[:], start=True, stop=False)
            nc.tensor.matmul(ps[:], akh, lk[:], start=False, stop=True)

            expv = work.tile([1, N], F16, tag="ex", bufs=4)
            sume = work.tile([1, 1], F32, tag="se", bufs=8)
            if shift:
                negm = work.tile([1, 1], F32, tag="negm", bufs=8)
                nc.vector.reduce_max(negm[:], ps[:], axis=AX.X, negate=True)
                nc.scalar.activation(expv[:], ps[:], AF.Exp, bias=negm[:],
                                     accum_out=sume[:])
            else:
                nc.scalar.activation(expv[:], ps[:], AF.Exp,
                                     accum_out=sume[:])
            if prev is not None and prev["b"] % 2 == 1:
                emit_alpha_dma(prev)  # on Act queue, after exp(b)
            rsum = work.tile([1, 1], F32, tag="rs", bufs=8)
            nc.vector.reciprocal(rsum[:], sume[:])
            rrep = work.tile([1, D], F16, tag="rr", bufs=4)
            nc.vector.tensor_scalar_mul(rrep[:], ones_sb[:], rsum[:])
            prev = dict(expv=expv, rrep=rrep, b=b)
            if b == 0:
                # flush batch 0 eagerly at max priority: its alpha opens
                # the write stream, so its latency is the kernel's head.
                with tc.high_priority():
                    emit_bcast(prev)
                    emit_flush(prev)
                    emit_alpha_dma(prev)
                prev = None

        emit_bcast(prev)
        emit_flush(prev)
        emit_alpha_dma(prev)

        # ---- bias (ema == 0): one tile, every output row identical -----
        v = work.tile([128, N], F32, tag="v", bufs=1)
        nc.vector.tensor_scalar_max(v[:], pbs_acc[:], EPS / MOM)
        bias_t = work.tile([128, N], F32, tag="biassb", bufs=1)
        nc.scalar.activation(bias_t[:], v[:], AF.Ln, scale=MOM)
        # 18/14 split: sync also carries alpha7 (1 MiB) in the tail, so the
        # scalar queue takes 18 of the 32 quarter-row chunks — both queues
        # then drain within ~0.1us of each other instead of 2.2us apart.
        dst = biaso.ap().rearrange("b (x p) j -> p (b x) j", p=128)
        src18 = bias_t[:].rearrange(
            "p (o n) -> p o n", o=1).broadcast_to([128, 18, N])
        src14 = bias_t[:].rearrange(
            "p (o n) -> p o n", o=1).broadcast_to([128, 14, N])
        nc.scalar.dma_start(dst[:, 0:18, :], src18)
        nc.sync.dma_start(dst[:, 18:32, :], src14)

    nc.compile()
    return nc


_NC_CACHE = {}


def _get_nc(zero_bias=True, shift=False):
    key = (zero_bias, shift)
    if key not in _NC_CACHE:
        _NC_CACHE[key] = build(zero_bias, shift)
    return _NC_CACHE[key]


def _needs_shift(X, W_fuse, b_fuse, W_q, W_k, a_q, a_k):
    """Exact host check: is 0.5*s inside the fp16-safe exp window?"""
    hi = 0.0
    for m in range(K):
        q = X @ (W_fuse @ W_q[m]) + b_fuse @ W_q[m]
        kk = X @ (W_fuse @ W_k[m]) + b_fuse @ W_k[m]
        s2 = 0.5 * (np.where(q > 0, q, 0.2 * q) @ a_q[m]
                    + np.where(kk > 0, kk, 0.2 * kk) @ a_k[m])
        hi = max(hi, float(np.abs(s2).max()))
    return hi > SAFE_HALF_S


def make_in_maps(desc_embeddings, name_value_embeddings, W_fuse, b_fuse,
                 W_q, W_k, a, alpha_ema, zero_bias=True):
    """Host-side sharding / weight prep -> per-core input dicts."""
    desc = np.asarray(desc_embeddings, np.float32)
    nve = np.asarray(name_value_embeddings, np.float32)
    W_fuse = np.asarray(W_fuse, np.float32)
    b_fuse = np.asarray(b_fuse, np.float32)
    W_q = np.asarray(W_q, np.float32)
    W_k = np.asarray(W_k, np.float32)
    a = np.asarray(a, np.float32)

    a_q = a[:, :R, 0]                      # [K,R]
    a_k = a[:, R:, 0]                      # [K,R]
    wq_f = np.einsum("cd,kdr->kcr", W_fuse, W_q)         # [K,2D,R]
    wk_f = np.einsum("cd,kdr->kcr", W_fuse, W_k)
    bq = np.einsum("d,kdr->kr", b_fuse, W_q)             # [K,R]
    bk = np.einsum("d,kdr->kr", b_fuse, W_k)

    # xT[b] fp16 [D, 2N]: partition d = [desc[b].T[d,:], nve[b].T[d,:]]
    xT = np.ascontiguousarray(
        np.stack([np.concatenate([desc[b].T, nve[b].T], axis=1)
                  for b in range(B)], axis=0).astype(np.float16))

    def lhsT(w):  # [2D, M] -> [D, 2M]: [:, h*M:(h+1)*M] = w[h*D:(h+1)*D]
        M = w.shape[1]
        return w.reshape(2, D, M).transpose(1, 0, 2).reshape(D, 2 * M)

    in_maps = []
    for m in range(N_CORES):
        wc = np.concatenate(
            [lhsT(wq_f[m]), lhsT(wk_f[m]),
             (0.5 * a_q[m]).reshape(D, 1), (0.5 * a_k[m]).reshape(D, 1)],
            axis=1).astype(np.float16)
        im = dict(xT=xT, wcat=np.ascontiguousarray(wc))
        if not zero_bias:
            im["bqk"] = np.ascontiguousarray(
                np.stack([bq[m], bk[m]], axis=1).astype(np.float32))
        in_maps.append(im)
    return in_maps


def gather(results, alpha_ema=None):
    alpha_full = np.stack([r["alpha"] for r in results], axis=1)
    ema = None if alpha_ema is None else np.asarray(alpha_ema, np.float32)
    if ema is not None and np.any(ema):
        # general-EMA fallback: exact host-side bias from device alpha
        new_ema = MOM * ema + (1.0 - MOM) * alpha_full.mean(axis=0)
        bias1 = np.log(np.maximum(new_ema, EPS))
        bias_full = np.ascontiguousarray(
            np.broadcast_to(bias1[None], (B, K, N, N)).astype(np.float32))
    else:
        bias_full = np.stack([r["bias"] for r in results], axis=1)
    return bias_full, alpha_full


def kernel(**inputs):
    desc = np.asarray(inputs["desc_embeddings"], np.float32)
    nve = np.asarray(inputs["name_value_embeddings"], np.float32)
    X = np.concatenate([desc, nve], axis=-1).reshape(B * N, 2 * D)
    a = np.asarray(inputs["a"], np.float32)
    zero_bias = not np.any(np.asarray(inputs["b_fuse"]))
    shift = _needs_shift(
        X, np.asarray(inputs["W_fuse"], np.float32),
        np.asarray(inputs["b_fuse"], np.float32),
        np.asarray(inputs["W_q"], np.float32),
        np.asarray(inputs["W_k"], np.float32),
        a[:, :R, 0], a[:, R:, 0])
    nc = _get_nc(zero_bias, shift)
    in_maps = make_in_maps(**inputs, zero_bias=zero_bias)
    res = run_bass_kernel_spmd(nc, in_maps, list(range(N_CORES)))
    return gather(res.results, inputs.get("alpha_ema"))
